# revision 46
# baseline (speedup 1.0000x reference)
"""Trainium2 Bass kernel for nn_AdversaryLayer_38723425140998.

RNN language-model layer: per step t (S=512 steps, B=256 batch, U=Z_K=256):
    h   = tanh(h_W[zsh_t] + h1_prev @ h_U + h_b)
    f,i = sigmoid(h @ {f,i}_W + b);  c = tanh(h @ c_W + b);  o = sigmoid(h @ o_W + b)
    h1  = h1_prev * f + c * i
    y_t = softmax(tanh((o * h1) @ t_W + t_b) @ y_W + y_b)

Strategy (8 NeuronCores):
  - Data-parallel: batch 256 -> 32 per core; weights replicated.
  - "Transposed space": state kept as h1^T [256 units (2x128 partitions), 32 batch].
    Weight matrices are the matmul stationary operand (bf16, fp32 PSUM accum);
    the moving operand is the narrow state (N=32).
  - Embedding: bf16 table (h_W + h_b) built on host; rows gathered via
    indirect DMA into [128-token, 256-unit] blocks that feed the PE directly
    as the stationary of the E-add matmul (the moving operand is an identity
    column slice selecting each step's 32 rows) -- no transpose DMAs and no
    resident E^T tile; the gather pool's buffer rotation paces gathers a few
    blocks ahead of the scan.
  - Fused software pipeline: the recurrent chain (hU -> tanh -> f,i,c -> h1) is
    the critical path; the o/t_W/y_W/softmax chain trails behind, processed in
    2-step pairs to halve its instruction count, and fills TensorE gaps.
  - The device keeps the hidden state doubled (H = 2*h1), which turns the
    whole sigmoid/gate tail into three fused scalar_tensor_tensor DVE ops per
    step with the residual 0.5/0.25 factors folded into h_U/o_W/t_W on the
    host; every ScalarE activation is a plain Tanh (or the per-pair Exp), so
    no Sigmoid LUT swap ever happens, and the softmax u8 quantization runs on
    GpSimd (idle engine).
  - I/O: per-call wall-clock through the axon relay is dominated by a ~1 ms
    per-tensor staging overhead plus bytes, so ALL inputs are packed into ONE
    u8 blob per core (weights host-converted to bf16 and pre-swizzled into the
    exact SBUF layouts; h_b folded into the embedding table) and both outputs
    into ONE u8 tensor: softmax rows leave the device as u8 codes against the
    row max (err <= 1/254 of the row max, ~4e-3 relative; gate is 2e-2)
    followed by one f32 scale per row. The host dequantizes. 1.07 MB in +
    4.25 MB out per core instead of 18 tensors / 2.4 MB in + 16.8 MB out.
  - Zero biases (the harness case) are detected at runtime and specialize the
    build: per-(gate,chunk) ACT bias instructions collapse into grouped ones.
  - bf16 everywhere except PSUM accumulation and the softmax (fp32).
"""
import os
import sys
from contextlib import ExitStack

for _p in ("/opt/trn_rl_repo", "/root/.axon_site/_ro/trn_rl_repo"):
    if os.path.isdir(_p) and _p not in sys.path:
        sys.path.insert(0, _p)

import numpy as np

import concourse.bass as bass
import concourse.tile as tile
from concourse.tile_rust import add_dep_helper
from concourse import bacc, mybir
from concourse.bass import IndirectOffsetOnAxis
from concourse.bass_utils import run_bass_kernel_spmd
from concourse.masks import make_identity

F32 = mybir.dt.float32
BF16 = mybir.dt.bfloat16
I32 = mybir.dt.int32
U8 = mybir.dt.uint8
AF = mybir.ActivationFunctionType

P = 128          # partitions
UC = 2           # unit chunks (256 units / 128)
ZK = 256         # vocab / output classes
U = 256          # hidden units
# 8 cores: the per-call wall-clock is dominated by the axon relay's per-call
# dispatch floor (~4 ms pipelined), under which the ~1.3 ms device makespan
# hides completely. A 4-core variant (B=64/core) lowers the dispatch floor to
# ~3.4 ms but its ~1.9 ms makespan pokes through; measured, the two tie within
# noise, and 8-core has the larger margin on device time.
N_CORES = 8
B_FULL = 256
S_FULL = 512
BL = B_FULL // N_CORES  # batch per core
TB = P // BL            # steps per embedding-gather block (TB*BL = 128 rows)
RQ = 2                  # softmax ring: quads per flush (8 steps)
NH = 2                  # independent phase-shifted half-batch chains
HB = BL // NH           # batch per half-chain


def _blob_layout(S, use_bias):
    """Byte (offset, size) of each segment in the packed input blob."""
    off, o = {}, 0

    def seg(key, n):
        nonlocal o
        off[key] = (o, n)
        o += n

    # tbl MUST stay at offset 0: indirect (gather) DMA requires a zero-offset
    # source AP.
    seg("tbl", (ZK + 1) * U * 2)        # bf16 h_W + h_b
    seg("z", BL * S * 4)                # int32 codes
    seg("w6", 6 * P * UC * U * 2)       # bf16 h_U,f,i,2*c,o,t pre-swizzled
    seg("wy", P * UC * ZK * 2)          # bf16 y_W pre-swizzled
    seg("h0", P * UC * 4)               # f32 initial hidden state
    if use_bias:
        seg("gb", 5 * P * UC * 4)       # f32 f/2, i/2, c, o/2, t biases
        seg("ybt", 4 * BL * ZK * 4)     # f32 y_b pre-broadcast over 4*BL rows
    return off, o


def build_kernel(S=S_FULL, use_bias=False, s_compute=None, chain="dve",
                 repeat=1, variant="full"):
    # timing-bisect variants (numerics are garbage for all but "full"/
    # "plaingather"): notail drops o/t/y/softmax; nofic also drops the
    # f,i,c matmuls + gate tanh (chain PE->Act->DVE); preonly keeps only
    # pa+tanh_h with a constant h1 (pure throughput, no recurrence);
    # noeadd drops the per-step E-add matmuls; plaingather uses plain
    # DMA loads instead of indirect gathers.
    do_tail = variant in ("full", "plaingather")
    do_fic = do_tail or variant in ("notail", "fic6", "fic24")
    do_dve = do_fic or variant == "nofic"
    indirect = variant != "plaingather"
    assert S % (TB * RQ) == 0
    if s_compute is None:
        s_compute = S
    nc = bacc.Bacc(None)

    off, nb_in = _blob_layout(S, use_bias)
    nb_y = BL * S * ZK
    nb_out = nb_y + BL * S * 4
    blob = nc.dram_tensor("blob", [nb_in], U8, kind="ExternalInput")
    out = nc.dram_tensor("out", [nb_out], U8, kind="ExternalOutput")

    def bv(key, dt, pattern=None, **axes):
        o, n = off[key]
        ap = blob[o:o + n].bitcast(dt)
        return ap.rearrange(pattern, **axes) if pattern else ap

    with tile.TileContext(nc) as tc, ExitStack() as ctx:
        # Timing builds (repeat > 1) double-buffer the per-repeat resident
        # tiles (weights, E, codes, scales) so repeat r+1's loads and
        # embedding gathers overlap repeat r's scan tail instead of
        # serializing on write-after-read; costs ~80 KB/partition extra SBUF.
        singles = ctx.enter_context(
            tc.tile_pool(name="singles", bufs=2 if repeat > 1 else 1))

        # ---- pools (created once; tiles rotate across repeats) ----------
        gthp = ctx.enter_context(tc.tile_pool(name="gth", bufs=32))
        psA = ctx.enter_context(tc.tile_pool(name="psA", bufs=2, space="PSUM"))
        psF = ctx.enter_context(tc.tile_pool(name="psF", bufs=3, space="PSUM"))
        psB = ctx.enter_context(tc.tile_pool(name="psB", bufs=2, space="PSUM"))
        psY = ctx.enter_context(tc.tile_pool(name="psY", bufs=1, space="PSUM"))
        p_preh = ctx.enter_context(tc.tile_pool(name="p_preh", bufs=3))
        p_h = ctx.enter_context(tc.tile_pool(name="p_h", bufs=3))
        p_fic = ctx.enter_context(tc.tile_pool(name="p_fic", bufs=3))
        p_o = ctx.enter_context(tc.tile_pool(name="p_o", bufs=3))
        p_h1 = ctx.enter_context(tc.tile_pool(name="p_h1", bufs=12))
        p_g = ctx.enter_context(tc.tile_pool(name="p_g", bufs=3))
        p_yr = ctx.enter_context(tc.tile_pool(name="p_yr", bufs=3))

        # Timing builds (repeat > 1) rerun the ENTIRE kernel body
        # back-to-back on device -- weight loads, gathers, scan, output
        # stores -- so wall-clock/repeat measures one full execution
        # without the axon relay's per-call dispatch floor. bufs=1
        # tiles serialize repeats via write-after-read dependencies.
        for _rep in range(repeat):
            # ---- weights -> bf16 SBUF chunk tiles (host pre-swizzled) ---------
            # wX[p, k, m, q]: stationary chunk (k, m) is wX[:, k, m, :] = W[128k+p, 128m+q]
            w6v = bv("w6", BF16, "(w p r) -> w p r", w=6, p=P)

            def load_w(i, name):
                t16 = singles.tile([P, UC, UC, P], BF16, tag=name)
                nc.sync.dma_start(
                    out=t16[:], in_=w6v[i].rearrange("p (k m q) -> p k m q", k=UC, q=P))
                return t16

            wu = load_w(0, "wu")
            wf = load_w(1, "wf")
            wi = load_w(2, "wi")
            wc = load_w(3, "wc")  # host pre-doubled: one Tanh(scale=0.5) serves f,i,c
            wo = load_w(4, "wo")
            wt = load_w(5, "wt")
            # y_W used as the moving operand: wy[:, k, :] = y_W[128k+p, :]
            wy = singles.tile([P, UC, ZK], BF16, tag="wy")
            nc.sync.dma_start(
                out=wy[:], in_=bv("wy", BF16, "(p k m) -> p k m", p=P, k=UC))

            # ---- biases (only loaded/applied when nonzero; host pre-scaled) ---
            if use_bias:
                gbv = bv("gb", F32, "(g p c) -> g p c", g=5, p=P)

                def load_b(gi, name):
                    t = singles.tile([P, UC], F32, tag=name)
                    nc.sync.dma_start(out=t[:], in_=gbv[gi])
                    return t

                fb = load_b(0, "fb")
                ib = load_b(1, "ib")
                cb = load_b(2, "cb")
                ob = load_b(3, "ob")
                tb_ = load_b(4, "tb")
                ybt = singles.tile([4 * BL, ZK], F32, tag="ybt")
                nc.sync.dma_start(
                    out=ybt[:], in_=bv("ybt", F32, "(b k) -> b k", b=4 * BL))

            # identity stationary for accumulating E^T into PSUM via TensorE
            ident = singles.tile([P, P], BF16, tag="ident")
            make_identity(nc, ident[:])

            # ---- initial hidden state (h0 broadcast over batch) --------------
            h0t = singles.tile([P, UC, 1], F32, tag="h0t")
            nc.sync.dma_start(out=h0t[:, :, 0], in_=bv("h0", F32, "(p c) -> p c", p=P))
            h1i32 = singles.tile([P, UC, BL], F32, tag="h1i32")
            nc.vector.memset(h1i32[:], 0.0)
            for c in range(UC):
                nc.vector.tensor_scalar_add(h1i32[:, c, :], h1i32[:, c, :], h0t[:, c, 0:1])
            h1i = [singles.tile([P, UC, HB], BF16, name=f"h1i{hb}", tag=f"h1i{hb}")
                   for hb in range(NH)]
            for hb in range(NH):
                nc.vector.tensor_copy(
                    out=h1i[hb][:], in_=h1i32[:, :, hb * HB:(hb + 1) * HB])

            # ---- shifted codes: zsh[b, 0] = 0, zsh[b, t] = z[b, t-1] + 1 -----
            zt = singles.tile([BL, S], I32, tag="zt")
            nc.sync.dma_start(out=zt[:], in_=bv("z", I32, "(b s) -> b s", b=BL))
            zsh = singles.tile([BL, S], I32, tag="zsh")
            nc.vector.memset(zsh[:, 0:1], 0)
            nc.vector.tensor_scalar_add(zsh[:, 1:S], zt[:, 0:S - 1], 1)

            # swizzle to gather order: zsw[s*BL + b, blk] = zsh[b, blk*TB + s]
            n_blocks = S // TB
            zsw = singles.tile([P, n_blocks], I32, tag="zsw")
            zsh_v = zsh[:].rearrange("b (blk s) -> b blk s", s=TB)
            for s in range(TB):
                nc.sync.dma_start(out=zsw[s * BL:(s + 1) * BL, :], in_=zsh_v[:, :, s])

            # ---- embedding gather ---------------------------------------------
            # Grouped: one indirect DMA fetches GG blocks of 128 token rows,
            # gt[p, j, u] = table_bf16[zsw[p, g*GG+j], u]. No transpose DMAs
            # and no resident E^T: each gathered block feeds the PE directly
            # as the stationary of the E-add matmul; the pool rotation (bufs)
            # paces gathers GG*TB*bufs steps ahead of the scan that reads them.
            # NOTE: gathering multiple blocks per indirect DMA (offset AP
            # [128, G]) matches CoreSim but returns garbage on hardware --
            # keep one 128-row block per gather.
            tblv = bv("tbl", BF16, "(r u) -> r u", u=U)
            gth_of = {}
            for blk in range(n_blocks):
                gt = gthp.tile([P, 1, U], BF16, name="gt", tag="gt")
                if indirect:
                    nc.gpsimd.indirect_dma_start(
                        out=gt[:, 0, :], out_offset=None, in_=tblv,
                        in_offset=IndirectOffsetOnAxis(ap=zsw[:, blk:blk + 1], axis=0))
                else:
                    nc.sync.dma_start(out=gt[:, 0, :], in_=tblv[0:P, :])
                gth_of[blk] = (gt, 0)


            # State convention: H = 2*h1 ("doubled" hidden state). Then
            #   H = 0.5*[(ft+1) o H_prev] + (it+1) o c,  ft/it = tanh(pre/2),
            # which needs only three fused scalar_tensor_tensor DVE ops and no
            # separate sigmoid fixup. The 1/2 of h1 = H/2 is folded into h_U (and
            # the o-gate's (ot+1)*H = 4*g into t_W) on the host.
            #
            # Quad structure: QD=4 steps = one gather block. Per quad the
            # pre_h PSUM tile is seeded ONCE with E^T for all 4 steps (two
            # N=128 matmuls instead of eight N=32 ones: stationary loads
            # dominate real PE time), and the o/t_W/y/softmax tail runs once
            # per quad at 4x width, quartering its instruction count.
            QD = TB                 # 4 steps per quad (= gather block)
            NQ = S // QD
            NQC = s_compute // QD
            h1_prev = h1i
            h1_hist = {}    # step -> H tile (o-gate needs H for each quad slot)
            h_ring = {}     # quad -> h^T ring tile [P, UC, QD, BL]
            g_ring = {}     # quad -> g^T ring tile [P, UC, QD, BL]
            psB_of = {}     # quad -> psum tile [P, 4, QD*BL]: o m0,m1 | t m0,m1
            tt_of = {}      # quad -> tt^T tile [P, UC, QD, BL]
            pa_of = {}      # quad -> pre_h psum [P, UC, QD, BL] (E-seeded)
            q_ring = None   # [QD*BL, RQ, ZK] u8 sbuf (quantized softmax rows)
            # per-row dequant scale rowmax/(254*rowsum), column = quad index
            sfull = singles.tile([QD * BL, NQ], F32, tag="sfull")

            # output views: u8 codes then f32 scales
            yv_full = out[0:nb_y].rearrange("(b s k) -> b s k", b=BL, s=S)
            # scales: ys[b, QD*qj + s] = sfull[s*BL + b, qj]
            sv = out[nb_y:nb_out].bitcast(F32).rearrange(
                "(b q s) -> b s q", b=BL, s=QD)

            def flush_ring(last_q):
                """Store RQ quads of quantized softmax rows."""
                r0 = last_q - (RQ - 1)
                # q_ring[(s, b), r, k] -> y[b, QD*(r0+r) + s, k]
                t0 = QD * r0
                yv = yv_full[:, t0:t0 + QD * RQ, :].rearrange(
                    "b (r s) k -> b s r k", s=QD)
                for s in range(QD):
                    nc.sync.dma_start(
                        out=yv[:, s, :, :], in_=q_ring[s * BL:(s + 1) * BL, :, :])

            MUL = mybir.AluOpType.mult
            ADD = mybir.AluOpType.add
            # recurrent-tail elementwise ops live on the DVE; the hardware ISA
            # rejects scalar_tensor_tensor on gpsimd (cost model accepts it).
            ce = nc.gpsimd if chain == "gps" else nc.vector

            # The Tile scheduler is a greedy per-engine priority heap: a big
            # tail activation that becomes ready while the chain's next tanh
            # isn't yet blocks the in-order ScalarE for its full duration.
            # Ordering each tail activation after the SAME iteration's gate
            # tanh makes it start right when the chain enters its DVE/PE
            # segment (~800ns of ScalarE idle), where it fits.
            last_fic_act = None

            def after_chain(tail_inst):
                if last_fic_act is not None:
                    add_dep_helper(tail_inst.ins, last_fic_act.ins,
                                   reason="tail act after chain act")

            for t in range(s_compute + 4):
                if t < s_compute:
                    # -- recurrent critical path for step t --
                    q, sl = divmod(t, QD)
                    hp = tc.high_priority()
                    hp.__enter__()
                    if sl == 0:
                        # per-quad pre_h psum, seeded with E^T for all 4 steps:
                        # gth^T routes token rows through an identity moving
                        # operand (exact 1.0 weights, fp32 accum); column
                        # s*BL+b of chunk m is E^T[:, m] for (step s, batch b)
                        pa = psA.tile([P, UC, QD, BL], F32, name="paq", tag="paq")
                        pa_of[q] = pa
                        # start=True marks the WHOLE 2KB psum zero-region
                        # pending-zero, so only the FIRST seed matmul may
                        # set it; later matmuls first-touch-overwrite then
                        # accumulate via the per-byte pending bits.
                        gt_t, j_t = gth_of[q]
                        for m in range(UC):
                            nc.tensor.matmul(
                                out=pa[:, m, :, :],
                                lhsT=gt_t[:, j_t, m * P:(m + 1) * P],
                                rhs=ident[:], start=(m == 0), stop=False,
                                skip_group_check=True)
                        h_ring[q] = p_h.tile(
                            [P, UC, QD, BL], BF16, name="hr", tag="hr")
                    pa = pa_of[q]
                    hr = h_ring[q]
                    # Two independent half-batch chains (columns [0:16] and
                    # [16:32]), phase-shifted: on HW the chain is ~fully
                    # serial (every PE->Act->PE->Act->DVE hop pays its full
                    # drain+sem latency), so a second independent chain hides
                    # those latencies under the other half's engine work.
                    # Shared full-width pa/hr keep the o/t/y tail and the E
                    # seeding at full batch width; h1 is PER-HALF so chain A's
                    # next step never waits on chain B's h1 write (tile-level
                    # dependency tracking would otherwise re-synchronize them).
                    h1h = [p_h1.tile([P, UC, HB], BF16, name=f"h1_{hb}", tag=f"h1_{hb}")
                           for hb in range(NH)]
                    for hb in range(NH):
                        c0, c1 = hb * HB, (hb + 1) * HB
                        h1 = h1h[hb]
                        hp_ = h1_prev[hb]
                        for m in range(UC):
                            for k in range(UC):
                                nc.tensor.matmul(
                                    out=pa[:, m, sl, c0:c1], lhsT=wu[:, k, m, :],
                                    rhs=hp_[:, k, :],
                                    start=False, stop=(k == UC - 1),
                                    skip_group_check=True)
                        nc.scalar.activation(
                            out=hr[:, :, sl, c0:c1], in_=pa[:, :, sl, c0:c1],
                            func=AF.Tanh)

                        if not do_fic:
                            if do_dve:  # "nofic": serial PE->Act->DVE chain
                                ce.scalar_tensor_tensor(
                                    out=h1[:], in0=hr[:, :, sl, c0:c1],
                                    scalar=1.0, in1=hp_[:],
                                    op0=MUL, op1=MUL)
                            continue
                        pf = psF.tile([P, 3, UC, HB], F32, tag="pf")  # f | i | c
                        if variant == "fic6":  # k=0 only (wrong math)
                            for gi, wg in enumerate((wf, wi, wc)):
                                for m in range(UC):
                                    nc.tensor.matmul(
                                        out=pf[:, gi, m, :], lhsT=wg[:, 0, m, :],
                                        rhs=hr[:, 0, sl, c0:c1],
                                        start=True, stop=True)
                        elif variant == "fic24":  # N split 2x: double mms
                            for gi, wg in enumerate((wf, wi, wc)):
                                for m in range(UC):
                                    for hf in range(2):
                                        h0_, h1_ = hf * HB // 2, (hf + 1) * HB // 2
                                        for k in range(UC):
                                            nc.tensor.matmul(
                                                out=pf[:, gi, m, h0_:h1_],
                                                lhsT=wg[:, k, m, :],
                                                rhs=hr[:, k, sl, c0 + h0_:c0 + h1_],
                                                start=(k == 0), stop=(k == UC - 1))
                        else:
                            for gi, wg in enumerate((wf, wi, wc)):
                                for m in range(UC):
                                    for k in range(UC):
                                        nc.tensor.matmul(
                                            out=pf[:, gi, m, :], lhsT=wg[:, k, m, :],
                                            rhs=hr[:, k, sl, c0:c1],
                                            start=(k == 0), stop=(k == UC - 1))

                        fic = p_fic.tile([P, 3, UC, HB], BF16, tag="fic")
                        if use_bias:
                            for gi, bt in ((0, fb), (1, ib), (2, cb)):
                                for m in range(UC):
                                    fic_act = nc.scalar.activation(
                                        out=fic[:, gi, m, :], in_=pf[:, gi, m, :],
                                        func=AF.Tanh, scale=0.5, bias=bt[:, m:m + 1])
                        else:
                            fic_act = nc.scalar.activation(
                                out=fic[:], in_=pf[:], func=AF.Tanh, scale=0.5)
                        last_fic_act = fic_act

                        # H = 0.5*[(ft+1) o H_prev] + (it+1) o c
                        t1 = p_preh.tile([P, UC, HB], BF16, tag="t1")
                        ce.scalar_tensor_tensor(
                            out=t1[:], in0=fic[:, 0, :, :], scalar=1.0,
                            in1=hp_[:], op0=ADD, op1=MUL)
                        t2 = p_preh.tile([P, UC, HB], BF16, tag="t2")
                        ce.scalar_tensor_tensor(
                            out=t2[:], in0=fic[:, 1, :, :], scalar=1.0,
                            in1=fic[:, 2, :, :], op0=ADD, op1=MUL)
                        ce.scalar_tensor_tensor(
                            out=h1[:], in0=t1[:], scalar=0.5, in1=t2[:],
                            op0=MUL, op1=ADD)
                    hp.__exit__(None, None, None)
                    if do_tail:
                        h1_hist[t] = h1h

                    if sl == QD - 1 and do_tail:
                        # -- o for the completed quad (all 4 steps' h ready) --
                        pb = psB.tile([P, 4, QD * BL], F32, name="pb", tag="pb")
                        psB_of[q] = pb
                        for m in range(UC):
                            for k in range(UC):
                                nc.tensor.matmul(
                                    out=pb[:, m, :], lhsT=wo[:, k, m, :],
                                    rhs=hr[:, k, :, :].rearrange("p s b -> p (s b)"),
                                    start=(k == 0), stop=(k == UC - 1))
                        # w_o pre-halved on host: ot = tanh(pre_o/2) with scale=1
                        osb = p_o.tile([P, UC, QD, BL], BF16, tag="osb")
                        if use_bias:
                            for m in range(UC):
                                after_chain(nc.scalar.activation(
                                    out=osb[:, m, :, :], in_=pb[:, m, :],
                                    func=AF.Tanh, bias=ob[:, m:m + 1]))
                        else:
                            after_chain(nc.scalar.activation(
                                out=osb[:], in_=pb[:, 0:2, :], func=AF.Tanh))
                        # g' = (ot+1) o H = 4*(o o h1); the 1/4 is folded into t_W.
                        g_ring[q] = gr = p_g.tile(
                            [P, UC, QD, BL], BF16, name="gr", tag="gr")
                        for s in range(QD):
                            hs = h1_hist.pop(QD * q + s)
                            for hb in range(NH):
                                c0, c1 = hb * HB, (hb + 1) * HB
                                ce.scalar_tensor_tensor(
                                    out=gr[:, :, s, c0:c1],
                                    in0=osb[:, :, s, c0:c1], scalar=1.0,
                                    in1=hs[hb][:], op0=ADD, op1=MUL)
                    if do_dve:
                        h1_prev = h1h

                # -- t_W stage for quad (t-QD-1)//QD --
                if t % QD == 1 and t >= QD + 1 and do_tail:
                    q1 = (t - QD - 1) // QD
                    if q1 < NQC:
                        pb1 = psB_of.pop(q1)
                        gr1 = g_ring.pop(q1)
                        del h_ring[q1]
                        for m in range(UC):
                            for k in range(UC):
                                nc.tensor.matmul(
                                    out=pb1[:, 2 + m, :],
                                    lhsT=wt[:, k, m, :],
                                    rhs=gr1[:, k, :, :].rearrange("p s b -> p (s b)"),
                                    start=(k == 0), stop=(k == UC - 1))
                        tt = p_o.tile([P, UC, QD, BL], BF16, name="tt", tag="tt")
                        if use_bias:
                            for m in range(UC):
                                after_chain(nc.scalar.activation(
                                    out=tt[:, m, :, :], in_=pb1[:, 2 + m, :],
                                    func=AF.Tanh, bias=tb_[:, m:m + 1]))
                        else:
                            after_chain(nc.scalar.activation(
                                out=tt[:], in_=pb1[:, 2:4, :], func=AF.Tanh))
                        tt_of[q1] = tt

                # -- y stage for quad (t-QD-2)//QD --
                if t % QD == 2 and t >= QD + 2 and do_tail:
                    q2 = (t - QD - 2) // QD
                    if q2 < NQC:
                        tt2 = tt_of.pop(q2)
                        py = psY.tile([QD * BL, ZK], F32, tag="py")
                        for k in range(UC):
                            nc.tensor.matmul(
                                out=py[:],
                                lhsT=tt2[:, k, :, :].rearrange("p s b -> p (s b)"),
                                rhs=wy[:, k, :], start=(k == 0), stop=(k == UC - 1))
                        r = q2 % RQ
                        if r == 0:
                            q_ring = p_yr.tile(
                                [QD * BL, RQ, ZK], U8, name="qring", tag="qring")
                        yexp = p_yr.tile([QD * BL, ZK], F32, tag="yexp")
                        ysum = p_yr.tile([QD * BL, 1], F32, tag="ysum")
                        if use_bias:
                            ylog = p_yr.tile([QD * BL, ZK], F32, tag="ylog")
                            nc.vector.tensor_add(out=ylog[:], in0=py[:], in1=ybt[:])
                            after_chain(nc.scalar.activation(
                                out=yexp[:], in_=ylog[:], func=AF.Exp,
                                accum_out=ysum[:]))
                        else:
                            after_chain(nc.scalar.activation(
                                out=yexp[:], in_=py[:], func=AF.Exp,
                                accum_out=ysum[:]))
                        # u8 quantization: q = yexp * (254/rowmax) + 0.5 (conversion
                        # truncates; +0.5 also keeps q <= 255 under round-to-nearest)
                        ym = p_yr.tile([QD * BL, 1], F32, tag="ym")
                        nc.vector.reduce_max(
                            out=ym[:], in_=yexp[:], axis=mybir.AxisListType.X)
                        ym254 = p_yr.tile([QD * BL, 1], F32, tag="ym254")
                        nc.vector.tensor_scalar_mul(ym254[:], ym[:], 1.0 / 254.0)
                        rq = p_yr.tile([QD * BL, 1], F32, tag="rq")
                        nc.vector.reciprocal(out=rq[:], in_=ym254[:])
                        yrec = p_yr.tile([QD * BL, 1], F32, tag="yrec")
                        nc.vector.reciprocal(out=yrec[:], in_=ysum[:])
                        # host-side scale = rowmax/(254*rowsum)
                        nc.vector.tensor_mul(
                            out=sfull[:, q2:q2 + 1], in0=ym254[:], in1=yrec[:])
                        # quantize on GpSimd (idle engine; SBUF-only op)
                        nc.gpsimd.tensor_scalar(
                            q_ring[:, r, :], yexp[:], rq[:, 0:1], 0.5,
                            mybir.AluOpType.mult, mybir.AluOpType.add)
                        if r == RQ - 1:
                            flush_ring(q2)

            # scales out (after the scan)
            if do_tail:
                for s in range(QD):
                    nc.sync.dma_start(
                        out=sv[:, s, :], in_=sfull[s * BL:(s + 1) * BL, :])

    nc.finalize()
    return nc


_NC_CACHE = {}


def _get_nc(S, use_bias):
    key = (S, use_bias)
    if key not in _NC_CACHE:
        _NC_CACHE[key] = build_kernel(S, use_bias)
    return _NC_CACHE[key]


def _u8(a):
    return np.ascontiguousarray(a).view(np.uint8).reshape(-1)


def _pack_all(inputs):
    """Full inputs dict -> (use_bias, per-core in_maps for the blob kernel)."""
    import ml_dtypes

    f32 = lambda a: np.ascontiguousarray(np.asarray(a, dtype=np.float32))
    bf = lambda a: np.ascontiguousarray(a.astype(ml_dtypes.bfloat16))
    z = np.ascontiguousarray(np.asarray(inputs["z"], dtype=np.int32))
    inp = {k: f32(inputs[k]) for k in
           ("h_W", "h_U", "f_W", "i_W", "c_W", "o_W", "t_W", "y_W",
            "h_b", "f_b", "i_b", "c_b", "o_b", "t_b", "y_b")}
    h0 = f32(inputs["h0"]).reshape(1, U)
    use_bias = any(
        np.any(inp[k]) for k in ("f_b", "i_b", "c_b", "o_b", "t_b", "y_b"))

    # wX[p, k, m*128+q] = W[128k+p, 128m+q]. Scale folds (device keeps the
    # hidden state doubled, H = 2*h1, and uses plain tanh everywhere):
    #   h_U * 0.5   : h1_prev = H_prev/2
    #   c_W * 2     : one Tanh(scale=0.5) instruction serves ft, it and c
    #   o_W * 0.5   : ot = tanh(pre_o/2) with scale=1, mergeable with tt's Tanh
    #   t_W * 0.25  : the moving operand is g' = (ot+1) o H = 4*(o o h1)
    wstk = lambda W: bf(W).reshape(UC, P, U).transpose(1, 0, 2)
    parts = [
        _u8(bf(inp["h_W"] + inp["h_b"][None, :])),
        _u8(np.stack([wstk(0.5 * inp["h_U"]), wstk(inp["f_W"]), wstk(inp["i_W"]),
                      wstk(2.0 * inp["c_W"]), wstk(0.5 * inp["o_W"]),
                      wstk(0.25 * inp["t_W"])])),
        _u8(bf(inp["y_W"]).reshape(UC, P, ZK).transpose(1, 0, 2)),
        _u8(np.ascontiguousarray((2.0 * h0).reshape(UC, P).T)),
    ]
    if use_bias:
        # sigmoid(x+b) = 0.5*(1+tanh((x+b)/2)): pre-halve the sigmoid biases
        barr = lambda b, s: np.ascontiguousarray((b * s).reshape(UC, P).T)
        parts.append(_u8(np.stack([
            barr(inp["f_b"], 0.5), barr(inp["i_b"], 0.5), barr(inp["c_b"], 1.0),
            barr(inp["o_b"], 0.5), barr(inp["t_b"], 1.0)])))
        parts.append(_u8(np.tile(inp["y_b"][None, :], (4 * BL, 1))))
    tbl_u8 = parts[0]
    shared = np.concatenate(parts[1:])
    in_maps = [
        {"blob": np.concatenate([tbl_u8, _u8(z[c * BL:(c + 1) * BL, :]), shared])}
        for c in range(N_CORES)]
    return use_bias, in_maps


def _decode_out(out_bytes, S):
    """Per-core packed output -> f32 [BL, S, ZK] softmax rows."""
    nb_y = BL * S * ZK
    q = out_bytes[:nb_y].reshape(BL, S, ZK).astype(np.float32)
    sc = out_bytes[nb_y:].view(np.float32).reshape(BL, S)
    return q * sc[:, :, None]


def kernel(z, h_W, h_U, h_b, f_W, f_b, i_W, i_b, c_W, c_b,
           o_W, o_b, t_W, t_b, y_W, y_b, h0):
    z = np.asarray(z)
    B, S = z.shape
    inputs = dict(z=z, h_W=h_W, h_U=h_U, h_b=h_b, f_W=f_W, f_b=f_b, i_W=i_W,
                  i_b=i_b, c_W=c_W, c_b=c_b, o_W=o_W, o_b=o_b, t_W=t_W,
                  t_b=t_b, y_W=y_W, y_b=y_b, h0=h0)
    use_bias, in_maps = _pack_all(inputs)
    nc = _get_nc(S, use_bias)
    last_err = None
    for _attempt in range(4):
        try:
            res = run_bass_kernel_spmd(nc, in_maps, list(range(N_CORES)))
            break
        except Exception as e:  # transient NRT/device errors: retry
            last_err = e
            msg = str(e).upper()
            if "UNRECOVERABLE" not in msg and "UNAVAILABLE" not in msg:
                raise
            import time as _time
            _time.sleep(5 * (_attempt + 1))
            try:  # drop cached PJRT state so the retry reconnects cleanly
                import jax
                jax.clear_caches()
            except Exception:
                pass
    else:
        raise last_err
    return np.concatenate(
        [_decode_out(res.results[c]["out"], S) for c in range(N_CORES)], axis=0)


def _numpy_ref(inp):
    z = np.asarray(inp["z"]); B, S = z.shape
    zsh = np.concatenate([np.zeros((B, 1), np.int32), z[:, :-1] + 1], axis=1)
    sig = lambda x: 1 / (1 + np.exp(-x))
    h1 = np.repeat(np.asarray(inp["h0"]).reshape(1, U), B, axis=0).astype(np.float32)
    out = np.zeros((B, S, ZK), np.float32)
    for t in range(S):
        h = np.tanh(inp["h_W"][zsh[:, t]] + h1 @ inp["h_U"] + inp["h_b"])
        f = sig(h @ inp["f_W"] + inp["f_b"]); i = sig(h @ inp["i_W"] + inp["i_b"])
        c = np.tanh(h @ inp["c_W"] + inp["c_b"]); o = sig(h @ inp["o_W"] + inp["o_b"])
        h1 = h1 * f + c * i
        tt = np.tanh((o * h1) @ inp["t_W"] + inp["t_b"])
        lg = tt @ inp["y_W"] + inp["y_b"]
        e = np.exp(lg - lg.max(-1, keepdims=True))
        out[:, t, :] = e / e.sum(-1, keepdims=True)
    return out


if __name__ == "__main__":
    rng = np.random.default_rng(0)
    S = int(sys.argv[1]) if len(sys.argv) > 1 else 16
    zero_bias = len(sys.argv) > 2 and sys.argv[2] == "zero"
    g = lambda shape: (rng.standard_normal(shape) * 0.05).astype(np.float32)
    b = (lambda shape: np.zeros(shape, np.float32)) if zero_bias else g
    inputs = dict(
        z=rng.integers(0, ZK, (B_FULL, S)).astype(np.int32),
        h_W=g((ZK + 1, U)), h_U=g((U, U)), h_b=b((U,)),
        f_W=g((U, U)), f_b=b((U,)),
        i_W=g((U, U)), i_b=b((U,)),
        c_W=g((U, U)), c_b=b((U,)),
        o_W=g((U, U)), o_b=b((U,)),
        t_W=g((U, U)), t_b=b((U,)),
        y_W=g((U, ZK)), y_b=b((ZK,)),
        h0=(np.zeros((1, U), np.float32) if zero_bias
            else (rng.standard_normal((1, U)) * 0.05).astype(np.float32)))
    got = kernel(**inputs)
    exp = _numpy_ref(inputs)
    err = np.abs(got - exp)
    print(f"S={S} zero_bias={zero_bias}  absmax={err.max():.3e}  "
          f"(ref absmax {np.abs(exp).max():.3e})  rel={err.max() / np.abs(exp).max():.3e}")



# revision 57
# speedup vs baseline: 1.2517x; 1.2517x over previous
"""Trainium2 Bass kernel for nn_AdversaryLayer_38723425140998.

RNN language-model layer: per step t (S=512 steps, B=256 batch, U=Z_K=256):
    h   = tanh(h_W[zsh_t] + h1_prev @ h_U + h_b)
    f,i = sigmoid(h @ {f,i}_W + b);  c = tanh(h @ c_W + b);  o = sigmoid(h @ o_W + b)
    h1  = h1_prev * f + c * i
    y_t = softmax(tanh((o * h1) @ t_W + t_b) @ y_W + y_b)

Strategy (8 NeuronCores):
  - Data-parallel: batch 256 -> 32 per core; weights replicated.
  - "Transposed space": state kept as h1^T [256 units (2x128 partitions), 32 batch].
    Weight matrices are the matmul stationary operand (bf16, fp32 PSUM accum);
    the moving operand is the narrow state (N=32).
  - Embedding: bf16 table (h_W + h_b) built on host; rows gathered via
    indirect DMA into [128-token, 256-unit] blocks that feed the PE directly
    as the stationary of the E-add matmul (the moving operand is an identity
    column slice selecting each step's 32 rows) -- no transpose DMAs and no
    resident E^T tile; the gather pool's buffer rotation paces gathers a few
    blocks ahead of the scan.
  - Fused software pipeline: the recurrent chain (hU -> tanh -> f,i,c -> h1) is
    the critical path; the o/t_W/y_W/softmax chain trails behind, processed
    once per 4-step quad (quartering its instruction count), and fills engine
    gaps. Each quad's pre_h PSUM tile is seeded ONCE with E^T for all 4 steps
    (two N=128 matmuls instead of eight N=32 ones -- on real HW LDWEIGHTS
    costs ~27-53ns per 128-col stationary, unmodeled by the cost model).
    Only the quad's FIRST seed matmul may use start=True: start marks the
    whole 2KB PSUM zero-region pending-zero, so a second start would void the
    first seed's writes.
  - The recurrent chain runs as TWO independent phase-shifted half-batch
    chains (16+16 of the 32 rows): on HW every PE->Act->PE->Act->DVE hop pays
    its full pipeline-drain + semaphore latency, and a second independent
    chain hides part of that under the other half's engine work (~8% on HW).
  - Tail activations carry an explicit scheduler dependency on the same
    iteration's gate tanh ("after_chain"): the Tile scheduler is a greedy
    per-engine priority heap, and without the dep a 585ns softmax exp that
    becomes ready early blocks the in-order ScalarE right before the chain's
    next tanh needs it (~15% on HW).
  - The device keeps the hidden state doubled (H = 2*h1), which turns the
    whole sigmoid/gate tail into three fused scalar_tensor_tensor DVE ops per
    step with the residual 0.5/0.25 factors folded into h_U/o_W/t_W on the
    host; every ScalarE activation is a plain Tanh (or the per-pair Exp), so
    no Sigmoid LUT swap ever happens, and the softmax u8 quantization runs on
    GpSimd (idle engine).
  - I/O: per-call wall-clock through the axon relay is dominated by a ~1 ms
    per-tensor staging overhead plus bytes, so ALL inputs are packed into ONE
    u8 blob per core (weights host-converted to bf16 and pre-swizzled into the
    exact SBUF layouts; h_b folded into the embedding table) and both outputs
    into ONE u8 tensor: softmax rows leave the device as u8 codes against the
    row max (err <= 1/254 of the row max, ~4e-3 relative; gate is 2e-2)
    followed by one f32 scale per row. The host dequantizes. 1.07 MB in +
    4.25 MB out per core instead of 18 tensors / 2.4 MB in + 16.8 MB out.
  - Zero biases (the harness case) are detected at runtime and specialize the
    build: per-(gate,chunk) ACT bias instructions collapse into grouped ones.
  - bf16 everywhere except PSUM accumulation and the softmax (fp32).
"""
import os
import sys
from contextlib import ExitStack

for _p in ("/opt/trn_rl_repo", "/root/.axon_site/_ro/trn_rl_repo"):
    if os.path.isdir(_p) and _p not in sys.path:
        sys.path.insert(0, _p)

import numpy as np

import concourse.bass as bass
import concourse.tile as tile
from concourse.tile_rust import add_dep_helper
from concourse import bacc, mybir
from concourse.bass import IndirectOffsetOnAxis
from concourse.bass_utils import run_bass_kernel_spmd
from concourse.masks import make_identity

F32 = mybir.dt.float32
BF16 = mybir.dt.bfloat16
I32 = mybir.dt.int32
U8 = mybir.dt.uint8
AF = mybir.ActivationFunctionType

P = 128          # partitions
UC = 2           # unit chunks (256 units / 128)
ZK = 256         # vocab / output classes
U = 256          # hidden units
# 8 cores: the per-call wall-clock is dominated by the axon relay's per-call
# dispatch floor (~4 ms pipelined), under which the ~1.3 ms device makespan
# hides completely. A 4-core variant (B=64/core) lowers the dispatch floor to
# ~3.4 ms but its ~1.9 ms makespan pokes through; measured, the two tie within
# noise, and 8-core has the larger margin on device time.
N_CORES = 8
B_FULL = 256
S_FULL = 512
BL = B_FULL // N_CORES  # batch per core
TB = P // BL            # steps per embedding-gather block (TB*BL = 128 rows)
RQ = 2                  # softmax ring: quads per flush (8 steps)
NH = 2                  # independent phase-shifted half-batch chains
HB = BL // NH           # batch per half-chain


def _blob_layout(S, use_bias):
    """Byte (offset, size) of each segment in the packed input blob."""
    off, o = {}, 0

    def seg(key, n):
        nonlocal o
        off[key] = (o, n)
        o += n

    # tbl MUST stay at offset 0: indirect (gather) DMA requires a zero-offset
    # source AP.
    seg("tbl", (ZK + 1) * U * 2)        # bf16 h_W + h_b
    seg("z", BL * S * 4)                # int32 codes
    seg("w6", 6 * P * UC * U * 2)       # bf16 h_U,f,i,2*c,o,t pre-swizzled
    seg("wy", P * UC * ZK * 2)          # bf16 y_W pre-swizzled
    seg("h0", P * UC * 4)               # f32 initial hidden state
    if use_bias:
        seg("gb", 5 * P * UC * 4)       # f32 f/2, i/2, c, o/2, t biases
        seg("ybt", 4 * BL * ZK * 4)     # f32 y_b pre-broadcast over 4*BL rows
    return off, o


def build_kernel(S=S_FULL, use_bias=False, s_compute=None, chain="dve",
                 repeat=1, variant="full"):
    # timing-bisect variants (numerics are garbage for all but "full"/
    # "plaingather"): notail drops o/t/y/softmax; nofic also drops the
    # f,i,c matmuls + gate tanh (chain PE->Act->DVE); preonly keeps only
    # pa+tanh_h with a constant h1 (pure throughput, no recurrence);
    # noeadd drops the per-step E-add matmuls; plaingather uses plain
    # DMA loads instead of indirect gathers.
    do_tail = variant in ("full", "plaingather")
    do_fic = do_tail or variant in ("notail", "fic6", "fic24")
    do_dve = do_fic or variant == "nofic"
    indirect = variant != "plaingather"
    assert S % (TB * RQ) == 0
    if s_compute is None:
        s_compute = S
    nc = bacc.Bacc(None)

    off, nb_in = _blob_layout(S, use_bias)
    nb_y = BL * S * ZK
    nb_out = nb_y + BL * S * 4
    blob = nc.dram_tensor("blob", [nb_in], U8, kind="ExternalInput")
    out = nc.dram_tensor("out", [nb_out], U8, kind="ExternalOutput")

    def bv(key, dt, pattern=None, **axes):
        o, n = off[key]
        ap = blob[o:o + n].bitcast(dt)
        return ap.rearrange(pattern, **axes) if pattern else ap

    with tile.TileContext(nc) as tc, ExitStack() as ctx:
        # Timing builds (repeat > 1) double-buffer the per-repeat resident
        # tiles (weights, E, codes, scales) so repeat r+1's loads and
        # embedding gathers overlap repeat r's scan tail instead of
        # serializing on write-after-read; costs ~80 KB/partition extra SBUF.
        singles = ctx.enter_context(
            tc.tile_pool(name="singles", bufs=2 if repeat > 1 else 1))

        # ---- pools (created once; tiles rotate across repeats) ----------
        gthp = ctx.enter_context(tc.tile_pool(name="gth", bufs=32))
        psA = ctx.enter_context(tc.tile_pool(name="psA", bufs=2, space="PSUM"))
        psF = ctx.enter_context(tc.tile_pool(name="psF", bufs=3, space="PSUM"))
        psB = ctx.enter_context(tc.tile_pool(name="psB", bufs=2, space="PSUM"))
        psY = ctx.enter_context(tc.tile_pool(name="psY", bufs=1, space="PSUM"))
        p_preh = ctx.enter_context(tc.tile_pool(name="p_preh", bufs=3))
        p_h = ctx.enter_context(tc.tile_pool(name="p_h", bufs=3))
        p_fic = ctx.enter_context(tc.tile_pool(name="p_fic", bufs=3))
        p_o = ctx.enter_context(tc.tile_pool(name="p_o", bufs=3))
        p_h1 = ctx.enter_context(tc.tile_pool(name="p_h1", bufs=6))
        p_g = ctx.enter_context(tc.tile_pool(name="p_g", bufs=3))
        p_yr = ctx.enter_context(tc.tile_pool(name="p_yr", bufs=3))

        # Timing builds (repeat > 1) rerun the ENTIRE kernel body
        # back-to-back on device -- weight loads, gathers, scan, output
        # stores -- so wall-clock/repeat measures one full execution
        # without the axon relay's per-call dispatch floor. bufs=1
        # tiles serialize repeats via write-after-read dependencies.
        for _rep in range(repeat):
            # ---- weights -> bf16 SBUF chunk tiles (host pre-swizzled) ---------
            # wX[p, k, m, q]: stationary chunk (k, m) is wX[:, k, m, :] = W[128k+p, 128m+q]
            w6v = bv("w6", BF16, "(w p r) -> w p r", w=6, p=P)

            def load_w(i, name):
                t16 = singles.tile([P, UC, UC, P], BF16, tag=name)
                nc.sync.dma_start(
                    out=t16[:], in_=w6v[i].rearrange("p (k m q) -> p k m q", k=UC, q=P))
                return t16

            wu = load_w(0, "wu")
            wf = load_w(1, "wf")
            wi = load_w(2, "wi")
            wc = load_w(3, "wc")  # host pre-doubled: one Tanh(scale=0.5) serves f,i,c
            wo = load_w(4, "wo")
            wt = load_w(5, "wt")
            # y_W used as the moving operand: wy[:, k, :] = y_W[128k+p, :]
            wy = singles.tile([P, UC, ZK], BF16, tag="wy")
            nc.sync.dma_start(
                out=wy[:], in_=bv("wy", BF16, "(p k m) -> p k m", p=P, k=UC))

            # ---- biases (only loaded/applied when nonzero; host pre-scaled) ---
            if use_bias:
                gbv = bv("gb", F32, "(g p c) -> g p c", g=5, p=P)

                def load_b(gi, name):
                    t = singles.tile([P, UC], F32, tag=name)
                    nc.sync.dma_start(out=t[:], in_=gbv[gi])
                    return t

                fb = load_b(0, "fb")
                ib = load_b(1, "ib")
                cb = load_b(2, "cb")
                ob = load_b(3, "ob")
                tb_ = load_b(4, "tb")
                ybt = singles.tile([4 * BL, ZK], F32, tag="ybt")
                nc.sync.dma_start(
                    out=ybt[:], in_=bv("ybt", F32, "(b k) -> b k", b=4 * BL))

            # identity stationary for accumulating E^T into PSUM via TensorE
            ident = singles.tile([P, P], BF16, tag="ident")
            make_identity(nc, ident[:])

            # ---- initial hidden state (h0 broadcast over batch) --------------
            h0t = singles.tile([P, UC, 1], F32, tag="h0t")
            nc.sync.dma_start(out=h0t[:, :, 0], in_=bv("h0", F32, "(p c) -> p c", p=P))
            h1i32 = singles.tile([P, UC, BL], F32, tag="h1i32")
            nc.vector.memset(h1i32[:], 0.0)
            for c in range(UC):
                nc.vector.tensor_scalar_add(h1i32[:, c, :], h1i32[:, c, :], h0t[:, c, 0:1])
            h1i = singles.tile([P, UC, BL], BF16, tag="h1i")
            nc.vector.tensor_copy(out=h1i[:], in_=h1i32[:])

            # ---- shifted codes: zsh[b, 0] = 0, zsh[b, t] = z[b, t-1] + 1 -----
            zt = singles.tile([BL, S], I32, tag="zt")
            nc.sync.dma_start(out=zt[:], in_=bv("z", I32, "(b s) -> b s", b=BL))
            zsh = singles.tile([BL, S], I32, tag="zsh")
            nc.vector.memset(zsh[:, 0:1], 0)
            nc.vector.tensor_scalar_add(zsh[:, 1:S], zt[:, 0:S - 1], 1)

            # swizzle to gather order: zsw[s*BL + b, blk] = zsh[b, blk*TB + s]
            n_blocks = S // TB
            zsw = singles.tile([P, n_blocks], I32, tag="zsw")
            zsh_v = zsh[:].rearrange("b (blk s) -> b blk s", s=TB)
            for s in range(TB):
                nc.sync.dma_start(out=zsw[s * BL:(s + 1) * BL, :], in_=zsh_v[:, :, s])

            # ---- embedding gather ---------------------------------------------
            # Grouped: one indirect DMA fetches GG blocks of 128 token rows,
            # gt[p, j, u] = table_bf16[zsw[p, g*GG+j], u]. No transpose DMAs
            # and no resident E^T: each gathered block feeds the PE directly
            # as the stationary of the E-add matmul; the pool rotation (bufs)
            # paces gathers GG*TB*bufs steps ahead of the scan that reads them.
            # NOTE: gathering multiple blocks per indirect DMA (offset AP
            # [128, G]) matches CoreSim but returns garbage on hardware --
            # keep one 128-row block per gather.
            tblv = bv("tbl", BF16, "(r u) -> r u", u=U)
            gth_of = {}
            for blk in range(n_blocks):
                gt = gthp.tile([P, 1, U], BF16, name="gt", tag="gt")
                if indirect:
                    nc.gpsimd.indirect_dma_start(
                        out=gt[:, 0, :], out_offset=None, in_=tblv,
                        in_offset=IndirectOffsetOnAxis(ap=zsw[:, blk:blk + 1], axis=0))
                else:
                    nc.sync.dma_start(out=gt[:, 0, :], in_=tblv[0:P, :])
                gth_of[blk] = (gt, 0)


            # State convention: H = 2*h1 ("doubled" hidden state). Then
            #   H = 0.5*[(ft+1) o H_prev] + (it+1) o c,  ft/it = tanh(pre/2),
            # which needs only three fused scalar_tensor_tensor DVE ops and no
            # separate sigmoid fixup. The 1/2 of h1 = H/2 is folded into h_U (and
            # the o-gate's (ot+1)*H = 4*g into t_W) on the host.
            #
            # Quad structure: QD=4 steps = one gather block. Per quad the
            # pre_h PSUM tile is seeded ONCE with E^T for all 4 steps (two
            # N=128 matmuls instead of eight N=32 ones: stationary loads
            # dominate real PE time), and the o/t_W/y/softmax tail runs once
            # per quad at 4x width, quartering its instruction count.
            QD = TB                 # 4 steps per quad (= gather block)
            NQ = S // QD
            NQC = s_compute // QD
            h1_prev = h1i
            h1_hist = {}    # step -> H tile (o-gate needs H for each quad slot)
            h_ring = {}     # quad -> h^T ring tile [P, UC, QD, BL]
            g_ring = {}     # quad -> g^T ring tile [P, UC, QD, BL]
            psB_of = {}     # quad -> psum tile [P, 4, QD*BL]: o m0,m1 | t m0,m1
            tt_of = {}      # quad -> tt^T tile [P, UC, QD, BL]
            pa_of = {}      # quad -> pre_h psum [P, UC, QD, BL] (E-seeded)
            q_ring = None   # [QD*BL, RQ, ZK] u8 sbuf (quantized softmax rows)
            # per-row dequant scale rowmax/(254*rowsum), column = quad index
            sfull = singles.tile([QD * BL, NQ], F32, tag="sfull")

            # output views: u8 codes then f32 scales
            yv_full = out[0:nb_y].rearrange("(b s k) -> b s k", b=BL, s=S)
            # scales: ys[b, QD*qj + s] = sfull[s*BL + b, qj]
            sv = out[nb_y:nb_out].bitcast(F32).rearrange(
                "(b q s) -> b s q", b=BL, s=QD)

            def flush_ring(last_q):
                """Store RQ quads of quantized softmax rows."""
                r0 = last_q - (RQ - 1)
                # q_ring[(s, b), r, k] -> y[b, QD*(r0+r) + s, k]
                t0 = QD * r0
                yv = yv_full[:, t0:t0 + QD * RQ, :].rearrange(
                    "b (r s) k -> b s r k", s=QD)
                for s in range(QD):
                    nc.sync.dma_start(
                        out=yv[:, s, :, :], in_=q_ring[s * BL:(s + 1) * BL, :, :])

            MUL = mybir.AluOpType.mult
            ADD = mybir.AluOpType.add
            # recurrent-tail elementwise ops live on the DVE; the hardware ISA
            # rejects scalar_tensor_tensor on gpsimd (cost model accepts it).
            ce = nc.gpsimd if chain == "gps" else nc.vector

            # The Tile scheduler is a greedy per-engine priority heap: a big
            # tail activation that becomes ready while the chain's next tanh
            # isn't yet blocks the in-order ScalarE for its full duration.
            # Ordering each tail activation after the SAME iteration's gate
            # tanh makes it start right when the chain enters its DVE/PE
            # segment (~800ns of ScalarE idle), where it fits.
            last_fic_act = None
            last_h1 = None

            def after_chain(tail_inst):
                if last_fic_act is not None and \
                        os.environ.get("K_NO_AFTER_CHAIN") != "1":
                    add_dep_helper(tail_inst.ins, last_fic_act.ins,
                                   reason="tail act after chain act")

            def after_h1(tail_inst):
                # Schedule-shaping for tail DVE/PE work (opt-in: measured
                # slightly WORSE in sim and neutral-to-worse on HW, so off
                # by default; kept for experiments).
                if last_h1 is not None and \
                        os.environ.get("K_ORDER_DVE") == "1":
                    add_dep_helper(tail_inst.ins, last_h1.ins,
                                   reason="tail after chain h1")

            for t in range(s_compute + 4):
                if t < s_compute:
                    # -- recurrent critical path for step t --
                    q, sl = divmod(t, QD)
                    hp = tc.high_priority()
                    hp.__enter__()
                    if sl == 0:
                        # per-quad pre_h psum, seeded with E^T for all 4 steps:
                        # gth^T routes token rows through an identity moving
                        # operand (exact 1.0 weights, fp32 accum); column
                        # s*BL+b of chunk m is E^T[:, m] for (step s, batch b)
                        pa = psA.tile([P, UC, QD, BL], F32, name="paq", tag="paq")
                        pa_of[q] = pa
                        # start=True marks the WHOLE 2KB psum zero-region
                        # pending-zero, so only the FIRST seed matmul may
                        # set it; later matmuls first-touch-overwrite then
                        # accumulate via the per-byte pending bits.
                        gt_t, j_t = gth_of[q]
                        for m in range(UC):
                            nc.tensor.matmul(
                                out=pa[:, m, :, :],
                                lhsT=gt_t[:, j_t, m * P:(m + 1) * P],
                                rhs=ident[:], start=(m == 0), stop=False,
                                skip_group_check=True)
                        h_ring[q] = p_h.tile(
                            [P, UC, QD, BL], BF16, name="hr", tag="hr")
                    pa = pa_of[q]
                    hr = h_ring[q]
                    # Two independent half-batch chains (columns [0:16] and
                    # [16:32]), phase-shifted: on HW the chain is ~fully
                    # serial (every PE->Act->PE->Act->DVE hop pays its full
                    # drain+sem latency), so a second independent chain hides
                    # those latencies under the other half's engine work.
                    # Shared full-width pa/hr keep the o/t/y tail and the E
                    # seeding at full batch width; h1 is PER-HALF so chain A's
                    # next step never waits on chain B's h1 write (tile-level
                    # dependency tracking would otherwise re-synchronize them).
                    h1 = p_h1.tile([P, UC, BL], BF16, tag="h1")
                    for hb in range(NH):
                        c0, c1 = hb * HB, (hb + 1) * HB
                        for m in range(UC):
                            for k in range(UC):
                                nc.tensor.matmul(
                                    out=pa[:, m, sl, c0:c1], lhsT=wu[:, k, m, :],
                                    rhs=h1_prev[:, k, c0:c1],
                                    start=False, stop=(k == UC - 1),
                                    skip_group_check=True)
                        nc.scalar.activation(
                            out=hr[:, :, sl, c0:c1], in_=pa[:, :, sl, c0:c1],
                            func=AF.Tanh)

                        if not do_fic:
                            if do_dve:  # "nofic": serial PE->Act->DVE chain
                                ce.scalar_tensor_tensor(
                                    out=h1[:, :, c0:c1], in0=hr[:, :, sl, c0:c1],
                                    scalar=1.0, in1=h1_prev[:, :, c0:c1],
                                    op0=MUL, op1=MUL)
                            continue
                        pf = psF.tile([P, 3, UC, HB], F32, tag="pf")  # f | i | c
                        if variant == "fic6":  # k=0 only (wrong math)
                            for gi, wg in enumerate((wf, wi, wc)):
                                for m in range(UC):
                                    nc.tensor.matmul(
                                        out=pf[:, gi, m, :], lhsT=wg[:, 0, m, :],
                                        rhs=hr[:, 0, sl, c0:c1],
                                        start=True, stop=True)
                        elif variant == "fic24":  # N split 2x: double mms
                            for gi, wg in enumerate((wf, wi, wc)):
                                for m in range(UC):
                                    for hf in range(2):
                                        h0_, h1_ = hf * HB // 2, (hf + 1) * HB // 2
                                        for k in range(UC):
                                            nc.tensor.matmul(
                                                out=pf[:, gi, m, h0_:h1_],
                                                lhsT=wg[:, k, m, :],
                                                rhs=hr[:, k, sl, c0 + h0_:c0 + h1_],
                                                start=(k == 0), stop=(k == UC - 1))
                        else:
                            for gi, wg in enumerate((wf, wi, wc)):
                                for m in range(UC):
                                    for k in range(UC):
                                        nc.tensor.matmul(
                                            out=pf[:, gi, m, :], lhsT=wg[:, k, m, :],
                                            rhs=hr[:, k, sl, c0:c1],
                                            start=(k == 0), stop=(k == UC - 1))

                        fic = p_fic.tile([P, 3, UC, HB], BF16, tag="fic")
                        if use_bias:
                            for gi, bt in ((0, fb), (1, ib), (2, cb)):
                                for m in range(UC):
                                    fic_act = nc.scalar.activation(
                                        out=fic[:, gi, m, :], in_=pf[:, gi, m, :],
                                        func=AF.Tanh, scale=0.5, bias=bt[:, m:m + 1])
                        else:
                            fic_act = nc.scalar.activation(
                                out=fic[:], in_=pf[:], func=AF.Tanh, scale=0.5)
                        last_fic_act = fic_act

                        # H = 0.5*[(ft+1) o H_prev] + (it+1) o c
                        t1 = p_preh.tile([P, UC, HB], BF16, tag="t1")
                        ce.scalar_tensor_tensor(
                            out=t1[:], in0=fic[:, 0, :, :], scalar=1.0,
                            in1=h1_prev[:, :, c0:c1], op0=ADD, op1=MUL)
                        t2 = p_preh.tile([P, UC, HB], BF16, tag="t2")
                        ce.scalar_tensor_tensor(
                            out=t2[:], in0=fic[:, 1, :, :], scalar=1.0,
                            in1=fic[:, 2, :, :], op0=ADD, op1=MUL)
                        last_h1 = ce.scalar_tensor_tensor(
                            out=h1[:, :, c0:c1], in0=t1[:], scalar=0.5, in1=t2[:],
                            op0=MUL, op1=ADD)
                    hp.__exit__(None, None, None)
                    if do_tail:
                        h1_hist[t] = h1

                    if sl == QD - 1 and do_tail:
                        # -- o for the completed quad (all 4 steps' h ready) --
                        pb = psB.tile([P, 4, QD * BL], F32, name="pb", tag="pb")
                        psB_of[q] = pb
                        for m in range(UC):
                            for k in range(UC):
                                mm = nc.tensor.matmul(
                                    out=pb[:, m, :], lhsT=wo[:, k, m, :],
                                    rhs=hr[:, k, :, :].rearrange("p s b -> p (s b)"),
                                    start=(k == 0), stop=(k == UC - 1))
                                if m == 0 and k == 0:
                                    after_h1(mm)
                        # w_o pre-halved on host: ot = tanh(pre_o/2) with scale=1
                        osb = p_o.tile([P, UC, QD, BL], BF16, tag="osb")
                        if use_bias:
                            for m in range(UC):
                                after_chain(nc.scalar.activation(
                                    out=osb[:, m, :, :], in_=pb[:, m, :],
                                    func=AF.Tanh, bias=ob[:, m:m + 1]))
                        else:
                            after_chain(nc.scalar.activation(
                                out=osb[:], in_=pb[:, 0:2, :], func=AF.Tanh))
                        # g' = (ot+1) o H = 4*(o o h1); the 1/4 is folded into t_W.
                        g_ring[q] = gr = p_g.tile(
                            [P, UC, QD, BL], BF16, name="gr", tag="gr")
                        for s in range(QD):
                            after_h1(ce.scalar_tensor_tensor(
                                out=gr[:, :, s, :], in0=osb[:, :, s, :], scalar=1.0,
                                in1=h1_hist.pop(QD * q + s)[:], op0=ADD, op1=MUL))
                    if do_dve:
                        h1_prev = h1

                # -- t_W stage for quad (t-QD-1)//QD --
                if t % QD == 1 and t >= QD + 1 and do_tail:
                    q1 = (t - QD - 1) // QD
                    if q1 < NQC:
                        pb1 = psB_of.pop(q1)
                        gr1 = g_ring.pop(q1)
                        del h_ring[q1]
                        for m in range(UC):
                            for k in range(UC):
                                mm = nc.tensor.matmul(
                                    out=pb1[:, 2 + m, :],
                                    lhsT=wt[:, k, m, :],
                                    rhs=gr1[:, k, :, :].rearrange("p s b -> p (s b)"),
                                    start=(k == 0), stop=(k == UC - 1))
                                if m == 0 and k == 0:
                                    after_h1(mm)
                        tt = p_o.tile([P, UC, QD, BL], BF16, name="tt", tag="tt")
                        if use_bias:
                            for m in range(UC):
                                after_chain(nc.scalar.activation(
                                    out=tt[:, m, :, :], in_=pb1[:, 2 + m, :],
                                    func=AF.Tanh, bias=tb_[:, m:m + 1]))
                        else:
                            after_chain(nc.scalar.activation(
                                out=tt[:], in_=pb1[:, 2:4, :], func=AF.Tanh))
                        tt_of[q1] = tt

                # -- y stage for quad (t-QD-2)//QD --
                if t % QD == 2 and t >= QD + 2 and do_tail:
                    q2 = (t - QD - 2) // QD
                    if q2 < NQC:
                        tt2 = tt_of.pop(q2)
                        py = psY.tile([QD * BL, ZK], F32, tag="py")
                        for k in range(UC):
                            mm = nc.tensor.matmul(
                                out=py[:],
                                lhsT=tt2[:, k, :, :].rearrange("p s b -> p (s b)"),
                                rhs=wy[:, k, :], start=(k == 0), stop=(k == UC - 1))
                            if k == 0:
                                after_h1(mm)
                        r = q2 % RQ
                        if r == 0:
                            q_ring = p_yr.tile(
                                [QD * BL, RQ, ZK], U8, name="qring", tag="qring")
                        yexp = p_yr.tile([QD * BL, ZK], F32, tag="yexp")
                        ysum = p_yr.tile([QD * BL, 1], F32, tag="ysum")
                        if use_bias:
                            ylog = p_yr.tile([QD * BL, ZK], F32, tag="ylog")
                            nc.vector.tensor_add(out=ylog[:], in0=py[:], in1=ybt[:])
                            after_chain(nc.scalar.activation(
                                out=yexp[:], in_=ylog[:], func=AF.Exp,
                                accum_out=ysum[:]))
                        else:
                            after_chain(nc.scalar.activation(
                                out=yexp[:], in_=py[:], func=AF.Exp,
                                accum_out=ysum[:]))
                        # u8 quantization: q = yexp * (254/rowmax) + 0.5 (conversion
                        # truncates; +0.5 also keeps q <= 255 under round-to-nearest)
                        ym = p_yr.tile([QD * BL, 1], F32, tag="ym")
                        after_h1(nc.vector.reduce_max(
                            out=ym[:], in_=yexp[:], axis=mybir.AxisListType.X))
                        ym254 = p_yr.tile([QD * BL, 1], F32, tag="ym254")
                        nc.vector.tensor_scalar_mul(ym254[:], ym[:], 1.0 / 254.0)
                        rq = p_yr.tile([QD * BL, 1], F32, tag="rq")
                        nc.vector.reciprocal(out=rq[:], in_=ym254[:])
                        yrec = p_yr.tile([QD * BL, 1], F32, tag="yrec")
                        nc.vector.reciprocal(out=yrec[:], in_=ysum[:])
                        # host-side scale = rowmax/(254*rowsum)
                        nc.vector.tensor_mul(
                            out=sfull[:, q2:q2 + 1], in0=ym254[:], in1=yrec[:])
                        # quantize on GpSimd (idle engine; SBUF-only op)
                        nc.gpsimd.tensor_scalar(
                            q_ring[:, r, :], yexp[:], rq[:, 0:1], 0.5,
                            mybir.AluOpType.mult, mybir.AluOpType.add)
                        if r == RQ - 1:
                            flush_ring(q2)

            # scales out (after the scan)
            if do_tail:
                for s in range(QD):
                    nc.sync.dma_start(
                        out=sv[:, s, :], in_=sfull[s * BL:(s + 1) * BL, :])

    nc.finalize()
    return nc


_NC_CACHE = {}


def _get_nc(S, use_bias):
    key = (S, use_bias)
    if key not in _NC_CACHE:
        _NC_CACHE[key] = build_kernel(S, use_bias)
    return _NC_CACHE[key]


def _u8(a):
    return np.ascontiguousarray(a).view(np.uint8).reshape(-1)


def _pack_all(inputs):
    """Full inputs dict -> (use_bias, per-core in_maps for the blob kernel)."""
    import ml_dtypes

    f32 = lambda a: np.ascontiguousarray(np.asarray(a, dtype=np.float32))
    bf = lambda a: np.ascontiguousarray(a.astype(ml_dtypes.bfloat16))
    z = np.ascontiguousarray(np.asarray(inputs["z"], dtype=np.int32))
    inp = {k: f32(inputs[k]) for k in
           ("h_W", "h_U", "f_W", "i_W", "c_W", "o_W", "t_W", "y_W",
            "h_b", "f_b", "i_b", "c_b", "o_b", "t_b", "y_b")}
    h0 = f32(inputs["h0"]).reshape(1, U)
    use_bias = any(
        np.any(inp[k]) for k in ("f_b", "i_b", "c_b", "o_b", "t_b", "y_b"))

    # wX[p, k, m*128+q] = W[128k+p, 128m+q]. Scale folds (device keeps the
    # hidden state doubled, H = 2*h1, and uses plain tanh everywhere):
    #   h_U * 0.5   : h1_prev = H_prev/2
    #   c_W * 2     : one Tanh(scale=0.5) instruction serves ft, it and c
    #   o_W * 0.5   : ot = tanh(pre_o/2) with scale=1, mergeable with tt's Tanh
    #   t_W * 0.25  : the moving operand is g' = (ot+1) o H = 4*(o o h1)
    wstk = lambda W: bf(W).reshape(UC, P, U).transpose(1, 0, 2)
    parts = [
        _u8(bf(inp["h_W"] + inp["h_b"][None, :])),
        _u8(np.stack([wstk(0.5 * inp["h_U"]), wstk(inp["f_W"]), wstk(inp["i_W"]),
                      wstk(2.0 * inp["c_W"]), wstk(0.5 * inp["o_W"]),
                      wstk(0.25 * inp["t_W"])])),
        _u8(bf(inp["y_W"]).reshape(UC, P, ZK).transpose(1, 0, 2)),
        _u8(np.ascontiguousarray((2.0 * h0).reshape(UC, P).T)),
    ]
    if use_bias:
        # sigmoid(x+b) = 0.5*(1+tanh((x+b)/2)): pre-halve the sigmoid biases
        barr = lambda b, s: np.ascontiguousarray((b * s).reshape(UC, P).T)
        parts.append(_u8(np.stack([
            barr(inp["f_b"], 0.5), barr(inp["i_b"], 0.5), barr(inp["c_b"], 1.0),
            barr(inp["o_b"], 0.5), barr(inp["t_b"], 1.0)])))
        parts.append(_u8(np.tile(inp["y_b"][None, :], (4 * BL, 1))))
    tbl_u8 = parts[0]
    shared = np.concatenate(parts[1:])
    in_maps = [
        {"blob": np.concatenate([tbl_u8, _u8(z[c * BL:(c + 1) * BL, :]), shared])}
        for c in range(N_CORES)]
    return use_bias, in_maps


def _decode_out(out_bytes, S):
    """Per-core packed output -> f32 [BL, S, ZK] softmax rows."""
    nb_y = BL * S * ZK
    q = out_bytes[:nb_y].reshape(BL, S, ZK).astype(np.float32)
    sc = out_bytes[nb_y:].view(np.float32).reshape(BL, S)
    return q * sc[:, :, None]


def kernel(z, h_W, h_U, h_b, f_W, f_b, i_W, i_b, c_W, c_b,
           o_W, o_b, t_W, t_b, y_W, y_b, h0):
    z = np.asarray(z)
    B, S = z.shape
    inputs = dict(z=z, h_W=h_W, h_U=h_U, h_b=h_b, f_W=f_W, f_b=f_b, i_W=i_W,
                  i_b=i_b, c_W=c_W, c_b=c_b, o_W=o_W, o_b=o_b, t_W=t_W,
                  t_b=t_b, y_W=y_W, y_b=y_b, h0=h0)
    use_bias, in_maps = _pack_all(inputs)
    nc = _get_nc(S, use_bias)
    last_err = None
    for _attempt in range(4):
        try:
            res = run_bass_kernel_spmd(nc, in_maps, list(range(N_CORES)))
            break
        except Exception as e:  # transient NRT/device errors: retry
            last_err = e
            msg = str(e).upper()
            if "UNRECOVERABLE" not in msg and "UNAVAILABLE" not in msg:
                raise
            import time as _time
            _time.sleep(5 * (_attempt + 1))
            try:  # drop cached PJRT state so the retry reconnects cleanly
                import jax
                jax.clear_caches()
            except Exception:
                pass
    else:
        raise last_err
    return np.concatenate(
        [_decode_out(res.results[c]["out"], S) for c in range(N_CORES)], axis=0)


def _numpy_ref(inp):
    z = np.asarray(inp["z"]); B, S = z.shape
    zsh = np.concatenate([np.zeros((B, 1), np.int32), z[:, :-1] + 1], axis=1)
    sig = lambda x: 1 / (1 + np.exp(-x))
    h1 = np.repeat(np.asarray(inp["h0"]).reshape(1, U), B, axis=0).astype(np.float32)
    out = np.zeros((B, S, ZK), np.float32)
    for t in range(S):
        h = np.tanh(inp["h_W"][zsh[:, t]] + h1 @ inp["h_U"] + inp["h_b"])
        f = sig(h @ inp["f_W"] + inp["f_b"]); i = sig(h @ inp["i_W"] + inp["i_b"])
        c = np.tanh(h @ inp["c_W"] + inp["c_b"]); o = sig(h @ inp["o_W"] + inp["o_b"])
        h1 = h1 * f + c * i
        tt = np.tanh((o * h1) @ inp["t_W"] + inp["t_b"])
        lg = tt @ inp["y_W"] + inp["y_b"]
        e = np.exp(lg - lg.max(-1, keepdims=True))
        out[:, t, :] = e / e.sum(-1, keepdims=True)
    return out


if __name__ == "__main__":
    rng = np.random.default_rng(0)
    S = int(sys.argv[1]) if len(sys.argv) > 1 else 16
    zero_bias = len(sys.argv) > 2 and sys.argv[2] == "zero"
    g = lambda shape: (rng.standard_normal(shape) * 0.05).astype(np.float32)
    b = (lambda shape: np.zeros(shape, np.float32)) if zero_bias else g
    inputs = dict(
        z=rng.integers(0, ZK, (B_FULL, S)).astype(np.int32),
        h_W=g((ZK + 1, U)), h_U=g((U, U)), h_b=b((U,)),
        f_W=g((U, U)), f_b=b((U,)),
        i_W=g((U, U)), i_b=b((U,)),
        c_W=g((U, U)), c_b=b((U,)),
        o_W=g((U, U)), o_b=b((U,)),
        t_W=g((U, U)), t_b=b((U,)),
        y_W=g((U, ZK)), y_b=b((ZK,)),
        h0=(np.zeros((1, U), np.float32) if zero_bias
            else (rng.standard_normal((1, U)) * 0.05).astype(np.float32)))
    got = kernel(**inputs)
    exp = _numpy_ref(inputs)
    err = np.abs(got - exp)
    print(f"S={S} zero_bias={zero_bias}  absmax={err.max():.3e}  "
          f"(ref absmax {np.abs(exp).max():.3e})  rel={err.max() / np.abs(exp).max():.3e}")

